# revision 16
# baseline (speedup 1.0000x reference)
"""Barrel shifter right 64 (zero-fill), batch 2097152, on 8 NeuronCores.

Layout: row-major. Each SBUF work tile holds 4096 rows: partition p carries 32
consecutive rows (spans), each span padded to 96 bf16 elements (32 zero guard +
64 data). A stage's shift-by-sa is a free-dim offset view whose low lanes read
the zero guard.

Engine split (DVE is the throughput limit — copy_predicated has no fast DVE
perf mode, so everything else moves off the vector engine):
  - ScalarE: f32->bf16 in-copy, the stage-0 shift-by-1 copy (odd bf16 offset),
    bf16->f32 out-copy.
  - DVE per stage: one 2x-mode tensor_scalar broadcast of the stage's raw
    shift bit into an int32 mask (no shift-amount decode needed — the inputs
    ARE the mux selects), and one 1x copy_predicated. Predicated copies run
    on int32 BF16 PAIRS (the mask is per row, so adjacent lanes share it;
    every shift >= 2 is pair-aligned), halving the 1x-mode element count.
    Stages >= 2 run in place with REVERSED innermost APs: processing high->low
    guarantees each shifted read (at k-w) happens before that position is
    overwritten, for any w, without the strip splitting forward order would
    need for sa=16/32.
  - Stage 0 (sa=1, odd bf16 offset, breaks pairs): ScalarE writes the shifted
    copy A->B, DVE predicates the unshifted A over it with the inverted mask
    (shift bit minus 1: nonzero exactly when the bit is clear).
"""

import sys

if "/opt/trn_rl_repo" not in sys.path:
    sys.path.insert(0, "/opt/trn_rl_repo")

import numpy as np

B_TOTAL = 2097152
NBITS = 64
NCTRL = 6
NCORES = 8
R_FULL = B_TOTAL // NCORES  # 262144 rows per core

P = 128
SPANS = 32                  # rows per partition per tile
TILE_ROWS = P * SPANS       # 4096
PITCH = 96                  # bf16 elems: guard(32) + bits(64)
GUARD = 32
W32 = NBITS // 2            # 32 int32 lanes per span
PITCH32 = PITCH // 2        # 48
GUARD32 = GUARD // 2        # 16
FD = SPANS * NBITS          # 2048
FD32 = SPANS * W32          # 1024
SFD = SPANS * NCTRL         # 192

_built = {}


def build(rows, sim_sync=False):
    # sim_sync inserts drains between same-engine dependent ops purely to
    # satisfy CoreSim's conservative OOO-engine race model; on hardware the
    # ops are all far above the ~266ns write-flush threshold (the proven
    # baseline relies on the same adjacency).
    import concourse.bass as bass
    from concourse import mybir

    f32 = mybir.dt.float32
    bf16 = mybir.dt.bfloat16
    i32 = mybir.dt.int32
    Alu = mybir.AluOpType

    nt = rows // TILE_ROWS
    assert rows % TILE_ROWS == 0

    nc = bass.Bass()
    data = nc.declare_dram_parameter("data", [rows, NBITS], f32, isOutput=False)
    shift = nc.declare_dram_parameter("shift", [rows, NCTRL], f32, isOutput=False)
    out = nc.declare_dram_parameter("out", [rows, NBITS], f32, isOutput=True)

    data_r = data.rearrange("(n p t) k -> n p (t k)", p=P, t=SPANS)
    shift_r = shift.rearrange("(n p t) k -> n p (t k)", p=P, t=SPANS)
    out_r = out.rearrange("(n p t) k -> n p (t k)", p=P, t=SPANS)

    def sb(name, shape, dt):
        return nc.alloc_sbuf_tensor(name, shape, dt)

    dtile = [sb(f"dtile{j}", [P, FD], f32) for j in (0, 1)]
    stile = [sb(f"stile{j}", [P, SFD], f32) for j in (0, 1)]
    # masks share the 48-pitch span structure of the work tiles so their APs
    # lower with the same dimension structure as copy_predicated's out/data
    msk = [sb(f"msk{j}", [P, SPANS * PITCH32], i32) for j in (0, 1)]
    wk = [sb(f"wk{j}", [P, SPANS * PITCH], bf16) for j in range(4)]
    otile = [sb(f"otile{j}", [P, FD], f32) for j in (0, 1)]

    def spans(t, off=GUARD):
        # [P, SPANS, NBITS] bf16 view at span-local offset `off`
        return t.ap().rearrange("p (t c) -> p t c", c=PITCH)[:, :, off:off + NBITS]

    def spans32(t, off=GUARD32, rev=False):
        # [P, SPANS, W32] int32 (bf16-pair) view at span-local int32 offset
        v = t.ap().bitcast(i32).rearrange("p (t c) -> p t c", c=PITCH32)[
            :, :, off:off + W32
        ]
        return v[:, :, ::-1] if rev else v

    with (
        nc.Block() as block,
        nc.semaphore("s_din0") as s_din0,
        nc.semaphore("s_din1") as s_din1,
        nc.semaphore("s_dout0") as s_dout0,
        nc.semaphore("s_dout1") as s_dout1,
        nc.semaphore("s_pre0") as s_pre0,
        nc.semaphore("s_pre1") as s_pre1,
        nc.semaphore("s_vec0") as s_vec0,
        nc.semaphore("s_vec1") as s_vec1,
        nc.semaphore("s_oc0") as s_oc0,
        nc.semaphore("s_oc1") as s_oc1,
        nc.semaphore("s_zero") as s_zero,
    ):
        s_din = [s_din0, s_din1]
        s_dout = [s_dout0, s_dout1]
        s_pre = [s_pre0, s_pre1]
        s_vec = [s_vec0, s_vec1]
        s_oc = [s_oc0, s_oc1]

        ntc = [(nt + 1) // 2, nt // 2]  # tiles per slot

        @block.sync
        def _(sp):
            for n in range(nt):
                c = n & 1
                k = n // 2
                if n >= 2:
                    # dtile free: ScalarE in-copy(n-2) done; stile free:
                    # DVE(n-2) done (masks read stile directly).
                    sp.wait_ge(s_pre[c], k)
                    sp.wait_ge(s_vec[c], k)
                sp.dma_start(
                    out=dtile[c].ap(), in_=data_r[n]
                ).then_inc(s_din[c], 16)
                sp.dma_start(
                    out=stile[c].ap(), in_=shift_r[n]
                ).then_inc(s_din[c], 16)
                if n >= 2:
                    # out-copy of tile n-2 (slot index k-1) raises s_oc to k
                    sp.wait_ge(s_oc[c], k)
                    sp.dma_start(
                        out=out_r[n - 2], in_=otile[c].ap()
                    ).then_inc(s_dout[c], 16)
            for n in (nt - 2, nt - 1):
                c = n & 1
                k = n // 2
                sp.wait_ge(s_oc[c], k + 1)
                sp.dma_start(
                    out=out_r[n], in_=otile[c].ap()
                ).then_inc(s_dout[c], 16)
            sp.wait_ge(s_dout0, 16 * ntc[0])
            sp.wait_ge(s_dout1, 16 * ntc[1])

        @block.scalar
        def _(s):
            s.wait_ge(s_zero, 1)
            for n in range(nt):
                c = n & 1
                k = n // 2
                # A_c (and B_c) free once DVE(n-2) is done with the slot
                if n >= 2:
                    s.wait_ge(s_vec[c], k)
                s.wait_ge(s_din[c], 32 * (k + 1))
                d3 = dtile[c].ap().rearrange("p (t k) -> p t k", k=NBITS)
                A, Bw = wk[2 * c], wk[2 * c + 1]
                s.copy(spans(A), d3)
                if n >= 2:
                    # tile n-2's out-copy doubles as write-flush spacing
                    # between the dependent in-copy and stage-0 copy
                    if n >= 4:
                        s.wait_ge(s_dout[c], 16 * (k - 1))
                    o3 = otile[c].ap().rearrange("p (t k) -> p t k", k=NBITS)
                    s.copy(o3, spans(Bw)).then_inc(s_oc[c], 1)
                elif sim_sync:
                    s.drain()
                # stage 0 shifted copy: B = A >> 1 lane (src reads one guard
                # zero at the span head)
                if sim_sync:
                    s.drain()
                s.copy(spans(Bw), spans(A, GUARD - 1)).then_inc(s_pre[c], 1)
            for n in (nt - 2, nt - 1):
                c = n & 1
                k = n // 2
                s.wait_ge(s_vec[c], k + 1)
                s.wait_ge(s_dout[c], 16 * k)  # otile[c] drained for tile n-2
                o3 = otile[c].ap().rearrange("p (t k) -> p t k", k=NBITS)
                s.copy(o3, spans(wk[2 * c + 1])).then_inc(s_oc[c], 1)

        @block.vector
        def _(v):
            # zero the work tiles once so every guard lane reads 0
            for j in range(4):
                ins = v.memset(wk[j].ap(), 0.0)
                if j == 3:
                    ins.then_inc(s_zero, 1)
            for n in range(nt):
                c = n & 1
                k = n // 2
                v.wait_ge(s_pre[c], k + 1)
                A, Bw = wk[2 * c], wk[2 * c + 1]
                st3 = stile[c].ap().rearrange("p (t j) -> p t j", j=NCTRL)
                m3 = msk[c].ap().rearrange("p (t k) -> p t k", k=PITCH32)[
                    :, :, 0:W32
                ]
                m3r = m3[:, :, ::-1]

                def bitbrd(i):
                    # stage-i select bit (shift[:, 5-i]) broadcast across the
                    # 32 int32 pair-lanes of its span
                    return st3[:, :, 5 - i:6 - i].broadcast_to([P, SPANS, W32])

                # stage 0: B already holds shift-by-1(A); predicate the
                # unshifted A over it where the select bit is CLEAR
                # (bit - 1 is nonzero exactly for bit == 0).
                v.tensor_scalar(m3, bitbrd(0), 1.0, None, Alu.subtract)
                if sim_sync:
                    v.drain()
                v.copy_predicated(spans32(Bw), m3, spans32(A))
                # stages 1..5 in place on B, reversed inner order: the read at
                # k-w always precedes the overwrite of k-w for any w.
                for i in range(1, 6):
                    w = (1 << i) // 2
                    if sim_sync:
                        v.drain()
                    v.tensor_copy(m3, bitbrd(i))
                    if sim_sync:
                        v.drain()
                    ins = v.copy_predicated(
                        spans32(Bw, rev=True),
                        m3r,
                        spans32(Bw, GUARD32 - w, rev=True),
                    )
                    if i == 5:
                        ins.then_inc(s_vec[c], 1)

    return nc


def _get(rows):
    if rows not in _built:
        _built[rows] = build(rows)
    return _built[rows]


def run_cores(data, shift, rows, trace=False):
    from concourse.bass_utils import run_bass_kernel_spmd

    nc = _get(rows)
    ncores = data.shape[0] // rows
    in_maps = [
        {
            "data": np.ascontiguousarray(data[i * rows:(i + 1) * rows]),
            "shift": np.ascontiguousarray(shift[i * rows:(i + 1) * rows]),
        }
        for i in range(ncores)
    ]
    res = run_bass_kernel_spmd(nc, in_maps, list(range(ncores)), trace=trace)
    full = np.concatenate([res.results[i]["out"] for i in range(ncores)], axis=0)
    return full, res


def kernel(data, shift):
    data = np.ascontiguousarray(np.asarray(data), dtype=np.float32)
    shift = np.ascontiguousarray(np.asarray(shift), dtype=np.float32)
    full, _ = run_cores(data, shift, R_FULL)
    return full.astype(np.float32, copy=False)


# revision 19
# speedup vs baseline: 1.0458x; 1.0458x over previous
"""Barrel shifter right 64 (zero-fill), batch 2097152, on 8 NeuronCores.

Layout: row-major. Each SBUF work tile holds 4096 rows: partition p carries 32
consecutive rows (spans), each span padded to 96 bf16 elements (32 zero guard +
64 data). A stage's shift-by-sa is a free-dim offset view whose low lanes read
the zero guard.

Engine split (DVE is the throughput limit — copy_predicated has no fast DVE
perf mode, so everything else moves off the vector engine):
  - ScalarE: f32->bf16 in-copy, the stage-0 shift-by-1 copy (odd bf16 offset),
    bf16->f32 out-copy.
  - DVE: per stage one 1x copy_predicated whose PREDICATE IS the raw f32
    shift bit broadcast across the span (copy_predicated tests nonzero; 1.0f
    != 0) — no mask materialization at all. Predicated copies run on int32
    BF16 PAIRS (the mask is per row, so adjacent lanes share it; every shift
    >= 2 is pair-aligned), halving the 1x-mode element count. Stages >= 2 run
    in place with REVERSED innermost APs: processing high->low guarantees
    each shifted read (at k-w) happens before that position is overwritten,
    for any w, without strip splitting.
  - Stage 0 (sa=1, odd bf16 offset, breaks pairs): ScalarE writes the shifted
    copy A->B, DVE predicates the unshifted A over it with the one
    materialized mask of the kernel: select bit minus 1 (nonzero exactly when
    the bit is clear).

Input tiles are triple-buffered so each tile's ~8.5us HBM load is issued ~2.5
tiles ahead of its DVE consumption.
"""

import sys

if "/opt/trn_rl_repo" not in sys.path:
    sys.path.insert(0, "/opt/trn_rl_repo")

import numpy as np

B_TOTAL = 2097152
NBITS = 64
NCTRL = 6
NCORES = 8
R_FULL = B_TOTAL // NCORES  # 262144 rows per core

P = 128
SPANS = 32                  # rows per partition per tile
TILE_ROWS = P * SPANS       # 4096
PITCH = 96                  # bf16 elems: guard(32) + bits(64)
GUARD = 32
W32 = NBITS // 2            # 32 int32 lanes per span
PITCH32 = PITCH // 2        # 48
GUARD32 = GUARD // 2        # 16
FD = SPANS * NBITS          # 2048
SFD = SPANS * NCTRL         # 192
NIN = 3                     # input-tile slots

_built = {}


def build(rows, sim_sync=False):
    # sim_sync inserts drains between same-engine dependent ops purely to
    # satisfy CoreSim's conservative OOO-engine race model; on hardware the
    # ops are all far above the ~266ns write-flush threshold (the proven
    # baseline relies on the same adjacency).
    import concourse.bass as bass
    from concourse import mybir

    f32 = mybir.dt.float32
    bf16 = mybir.dt.bfloat16
    i32 = mybir.dt.int32
    Alu = mybir.AluOpType

    nt = rows // TILE_ROWS
    assert rows % TILE_ROWS == 0

    nc = bass.Bass()
    data = nc.declare_dram_parameter("data", [rows, NBITS], f32, isOutput=False)
    shift = nc.declare_dram_parameter("shift", [rows, NCTRL], f32, isOutput=False)
    out = nc.declare_dram_parameter("out", [rows, NBITS], f32, isOutput=True)

    data_r = data.rearrange("(n p t) k -> n p (t k)", p=P, t=SPANS)
    shift_r = shift.rearrange("(n p t) k -> n p (t k)", p=P, t=SPANS)
    out_r = out.rearrange("(n p t) k -> n p (t k)", p=P, t=SPANS)

    def sb(name, shape, dt):
        return nc.alloc_sbuf_tensor(name, shape, dt)

    dtile = [sb(f"dtile{j}", [P, FD], f32) for j in range(NIN)]
    stile = [sb(f"stile{j}", [P, SFD], f32) for j in range(NIN)]
    # stage-0's inverted mask shares the 48-pitch span structure of the work
    # tiles so its AP lowers with the same dimension structure as
    # copy_predicated's out/data
    msk = sb("msk", [P, SPANS * PITCH32], i32)
    wk = [sb(f"wk{j}", [P, SPANS * PITCH], bf16) for j in range(4)]
    otile = [sb(f"otile{j}", [P, FD], f32) for j in (0, 1)]

    def spans(t, off=GUARD):
        # [P, SPANS, NBITS] bf16 view at span-local offset `off`
        return t.ap().rearrange("p (t c) -> p t c", c=PITCH)[:, :, off:off + NBITS]

    def spans32(t, off=GUARD32, rev=False):
        # [P, SPANS, W32] int32 (bf16-pair) view at span-local int32 offset
        v = t.ap().bitcast(i32).rearrange("p (t c) -> p t c", c=PITCH32)[
            :, :, off:off + W32
        ]
        return v[:, :, ::-1] if rev else v

    with (
        nc.Block() as block,
        nc.semaphore("s_din0") as s_din0,
        nc.semaphore("s_din1") as s_din1,
        nc.semaphore("s_din2") as s_din2,
        nc.semaphore("s_dout0") as s_dout0,
        nc.semaphore("s_dout1") as s_dout1,
        nc.semaphore("s_pre") as s_pre,
        nc.semaphore("s_vec") as s_vec,
        nc.semaphore("s_oc") as s_oc,
        nc.semaphore("s_zero") as s_zero,
    ):
        s_din = [s_din0, s_din1, s_din2]
        s_dout = [s_dout0, s_dout1]

        @block.sync
        def _(sp):
            for n in range(nt):
                c = n & 1
                r = n % NIN
                if n >= NIN:
                    # input slot r free once ScalarE's in-copy (dtile) and
                    # DVE (stile, read by the predicates) of tile n-NIN done
                    sp.wait_ge(s_pre, n - NIN + 1)
                    sp.wait_ge(s_vec, n - NIN + 1)
                sp.dma_start(
                    out=dtile[r].ap(), in_=data_r[n]
                ).then_inc(s_din[r], 16)
                sp.dma_start(
                    out=stile[r].ap(), in_=shift_r[n]
                ).then_inc(s_din[r], 16)
                if n >= 2:
                    sp.wait_ge(s_oc, n - 1)  # out-copy of tile n-2 done
                    sp.dma_start(
                        out=out_r[n - 2], in_=otile[c].ap()
                    ).then_inc(s_dout[c], 16)
            for n in (nt - 2, nt - 1):
                c = n & 1
                sp.wait_ge(s_oc, n + 1)
                sp.dma_start(
                    out=out_r[n], in_=otile[c].ap()
                ).then_inc(s_dout[c], 16)
            sp.wait_ge(s_dout0, 16 * ((nt + 1) // 2))
            sp.wait_ge(s_dout1, 16 * (nt // 2))

        @block.scalar
        def _(s):
            s.wait_ge(s_zero, 1)
            for n in range(nt):
                c = n & 1
                r = n % NIN
                # work-tile pair c free once DVE(n-2) is done
                if n >= 2:
                    s.wait_ge(s_vec, n - 1)
                s.wait_ge(s_din[r], 32 * (n // NIN + 1))
                d3 = dtile[r].ap().rearrange("p (t k) -> p t k", k=NBITS)
                A, Bw = wk[2 * c], wk[2 * c + 1]
                s.copy(spans(A), d3)
                if n >= 2:
                    # tile n-2's out-copy doubles as write-flush spacing
                    # between the dependent in-copy and stage-0 copy
                    if n >= 4:
                        s.wait_ge(s_dout[c], 16 * ((n - 4) // 2 + 1))
                    o3 = otile[c].ap().rearrange("p (t k) -> p t k", k=NBITS)
                    s.copy(o3, spans(Bw)).then_inc(s_oc, 1)
                elif sim_sync:
                    s.drain()
                # stage 0 shifted copy: B = A >> 1 lane (src reads one guard
                # zero at the span head)
                if sim_sync:
                    s.drain()
                s.copy(spans(Bw), spans(A, GUARD - 1)).then_inc(s_pre, 1)
            for n in (nt - 2, nt - 1):
                c = n & 1
                s.wait_ge(s_vec, n + 1)
                s.wait_ge(s_dout[c], 16 * ((n - 2) // 2 + 1))
                o3 = otile[c].ap().rearrange("p (t k) -> p t k", k=NBITS)
                s.copy(o3, spans(wk[2 * c + 1])).then_inc(s_oc, 1)

        @block.vector
        def _(v):
            # zero the work tiles once so every guard lane reads 0
            for j in range(4):
                ins = v.memset(wk[j].ap(), 0.0)
                if j == 3:
                    ins.then_inc(s_zero, 1)
            m3 = msk.ap().rearrange("p (t k) -> p t k", k=PITCH32)[:, :, 0:W32]
            for n in range(nt):
                c = n & 1
                r = n % NIN
                v.wait_ge(s_pre, n + 1)
                A, Bw = wk[2 * c], wk[2 * c + 1]
                st3 = stile[r].ap().rearrange("p (t j) -> p t j", j=NCTRL)
                # copy_predicated requires an integer-typed mask; the f32 bit
                # pattern of 1.0 is nonzero, so a bitcast view keeps semantics
                sti3 = stile[r].ap().bitcast(i32).rearrange(
                    "p (t j) -> p t j", j=NCTRL
                )

                def bitbrd(i, int_view=True):
                    # stage-i select bit (shift[:, 5-i]) broadcast across the
                    # 32 int32 pair-lanes of its span
                    src = sti3 if int_view else st3
                    return src[:, :, 5 - i:6 - i].broadcast_to([P, SPANS, W32])

                # stage 0: B already holds shift-by-1(A); predicate the
                # unshifted A over it where the select bit is CLEAR
                # (bit - 1 is nonzero exactly for bit == 0).
                v.tensor_scalar(m3, bitbrd(0, int_view=False), 1.0, None, Alu.subtract)
                if sim_sync:
                    v.drain()
                v.copy_predicated(spans32(Bw), m3, spans32(A))
                # stages 1..5 in place on B, reversed inner order (the read at
                # k-w always precedes the overwrite of k-w); the predicate is
                # the raw f32 select bit, broadcast — nonzero iff selected.
                for i in range(1, 6):
                    w = (1 << i) // 2
                    if sim_sync:
                        v.drain()
                    ins = v.copy_predicated(
                        spans32(Bw, rev=True),
                        bitbrd(i),
                        spans32(Bw, GUARD32 - w, rev=True),
                    )
                    if i == 5:
                        ins.then_inc(s_vec, 1)

    return nc


def _get(rows):
    if rows not in _built:
        _built[rows] = build(rows)
    return _built[rows]


def run_cores(data, shift, rows, trace=False):
    from concourse.bass_utils import run_bass_kernel_spmd

    nc = _get(rows)
    ncores = data.shape[0] // rows
    in_maps = [
        {
            "data": np.ascontiguousarray(data[i * rows:(i + 1) * rows]),
            "shift": np.ascontiguousarray(shift[i * rows:(i + 1) * rows]),
        }
        for i in range(ncores)
    ]
    res = run_bass_kernel_spmd(nc, in_maps, list(range(ncores)), trace=trace)
    full = np.concatenate([res.results[i]["out"] for i in range(ncores)], axis=0)
    return full, res


def kernel(data, shift):
    data = np.ascontiguousarray(np.asarray(data), dtype=np.float32)
    shift = np.ascontiguousarray(np.asarray(shift), dtype=np.float32)
    full, _ = run_cores(data, shift, R_FULL)
    return full.astype(np.float32, copy=False)
